# revision 1
# baseline (speedup 1.0000x reference)
"""GraphSAGE (2x SAGEConv mean-aggr + linear head + log_softmax) on 8 Trainium2
NeuronCores.

Strategy (graph/data parallel per the dst-sharding scheme):
  - Nodes are packed into 392 balanced bins (greedy by degree) -> 8 cores x 49
    windows x 128 slots. Edges are owned by their dst node's core.
  - Per 128-edge tile: indirect-DMA gather of source rows (x or h), then a
    one-hot selection matmul accumulates the weighted segment-sum directly in
    PSUM (S[e, j] = (dst_slot[e]==j) * 1/deg(dst[e])), i.e. PSUM window
    accumulates mean^T = sum_e w_e * x[src_e] per dst slot.
  - mean^T/x^T feed the SAGE linear layers as 128x128 matmuls in
    feature-major layout; relu via scalar engine with fused bias.
  - h1 rows are transposed back, written to DRAM, AllGathered across the 8
    cores, and layer 2 repeats against the gathered table. Head = matmul with
    Wlin + log_softmax over 2 logits.

kernel(**inputs) takes the full unsharded inputs and returns the full
[50000, 2] output; sharding/packing happens on the host inside this module.
"""
import sys
sys.path.insert(0, "/opt/trn_rl_repo")
import numpy as np

N = 50000
F = 128
NCORES = 8
NW = 49                 # windows per core
DPC = NW * 128          # dst slots per core (6272)
NBINS = NCORES * NW     # 392


# ----------------------------------------------------------------- host prep
def _pack(edge_index):
    """Assign nodes to (core, window, slot); build per-core edge tile arrays."""
    src = np.asarray(edge_index[0], dtype=np.int64)
    dst = np.asarray(edge_index[1], dtype=np.int64)
    deg = np.bincount(dst, minlength=N).astype(np.int64)
    inv = (1.0 / np.maximum(deg, 1)).astype(np.float32)

    # greedy: nodes by degree desc into the lightest bin with a free slot
    import heapq
    order = np.argsort(-deg, kind="stable")
    heap = [(0, b) for b in range(NBINS)]
    heapq.heapify(heap)
    bin_of = np.empty(N, dtype=np.int32)
    slot_of = np.empty(N, dtype=np.int32)
    bin_load = np.zeros(NBINS, dtype=np.int64)
    bin_used = np.zeros(NBINS, dtype=np.int32)
    for nidx in order:
        d = int(deg[nidx])
        while True:
            load, b = heapq.heappop(heap)
            if bin_used[b] < 128:
                break
        bin_of[nidx] = b
        slot_of[nidx] = bin_used[b]
        bin_used[b] += 1
        bin_load[b] += d
        if bin_used[b] < 128:
            heapq.heappush(heap, (load + d, b))

    # rank bins by load; rank r -> window r//8, core r%8 (equalizes window
    # loads across cores so the shared per-window tile count wastes little)
    rank = np.empty(NBINS, dtype=np.int64)
    rank[np.argsort(-bin_load, kind="stable")] = np.arange(NBINS)
    win_of_bin = (rank // NCORES).astype(np.int32)
    core_of_bin = (rank % NCORES).astype(np.int32)

    core_of = core_of_bin[bin_of]
    win_of = win_of_bin[bin_of]
    # global permuted position of each node in the AllGather'd table
    pos_of = core_of.astype(np.int64) * DPC + win_of.astype(np.int64) * 128 + slot_of

    # per (core, window) edge counts -> shared tiles-per-window
    e_core = core_of[dst]
    e_win = win_of[dst]
    cnt = np.zeros((NCORES, NW), dtype=np.int64)
    np.add.at(cnt, (e_core, e_win), 1)
    t_w = np.maximum(1, (cnt.max(axis=0) + 127) // 128).astype(np.int64)  # [NW]
    nt = int(t_w.sum())                       # tiles per core
    col0 = np.concatenate([[0], np.cumsum(t_w)])  # window -> first tile col

    # order edges by (core, window); fill tile arrays
    g1 = np.zeros((NCORES, 128, nt), dtype=np.int32)   # src node id
    g2 = np.zeros((NCORES, 128, nt), dtype=np.int32)   # perm pos of src
    dsl = np.zeros((NCORES, 128, nt), dtype=np.float32)  # dst slot in window
    wgt = np.zeros((NCORES, 128, nt), dtype=np.float32)  # 1/deg(dst), 0 = pad

    ekey = e_core.astype(np.int64) * NW + e_win
    eorder = np.argsort(ekey, kind="stable")
    sk = ekey[eorder]
    starts = np.searchsorted(sk, np.arange(NCORES * NW))
    ends = np.searchsorted(sk, np.arange(NCORES * NW) + 1)
    for k in range(NCORES):
        for j in range(NW):
            lo, hi = starts[k * NW + j], ends[k * NW + j]
            es = eorder[lo:hi]
            n = len(es)
            c0 = col0[j]
            p = np.arange(n) % 128
            t = np.arange(n) // 128
            g1[k, p, c0 + t] = src[es]
            g2[k, p, c0 + t] = pos_of[src[es]]
            dsl[k, p, c0 + t] = slot_of[dst[es]]
            wgt[k, p, c0 + t] = inv[dst[es]]

    return dict(g1=g1, g2=g2, dsl=dsl, wgt=wgt, t_w=t_w, col0=col0, nt=nt,
                core_of=core_of, win_of=win_of, slot_of=slot_of, pos_of=pos_of)


# --------------------------------------------------------------- bass build
def _build(t_w, col0, nt):
    import concourse.bass as bass
    import concourse.bacc as bacc
    import concourse.tile as tile
    from concourse import mybir

    f32 = mybir.dt.float32
    nc = bacc.Bacc("TRN2", target_bir_lowering=False, debug=False,
                   enable_asserts=False, num_devices=NCORES)
    x = nc.dram_tensor("x", [N, F], f32, kind="ExternalInput")
    xT = nc.dram_tensor("xT", [128, DPC], f32, kind="ExternalInput")
    g1 = nc.dram_tensor("g1", [128, nt], mybir.dt.int32, kind="ExternalInput")
    g2 = nc.dram_tensor("g2", [128, nt], mybir.dt.int32, kind="ExternalInput")
    dsl = nc.dram_tensor("dsl", [128, nt], f32, kind="ExternalInput")
    wgt = nc.dram_tensor("wgt", [128, nt], f32, kind="ExternalInput")
    w1l = nc.dram_tensor("w1l", [F, F], f32, kind="ExternalInput")
    w1r = nc.dram_tensor("w1r", [F, F], f32, kind="ExternalInput")
    w2l = nc.dram_tensor("w2l", [F, F], f32, kind="ExternalInput")
    w2r = nc.dram_tensor("w2r", [F, F], f32, kind="ExternalInput")
    b1 = nc.dram_tensor("b1", [F, 1], f32, kind="ExternalInput")
    b2 = nc.dram_tensor("b2", [F, 1], f32, kind="ExternalInput")
    wlin = nc.dram_tensor("wlin", [F, 2], f32, kind="ExternalInput")
    blinb = nc.dram_tensor("blinb", [128, 2], f32, kind="ExternalInput")
    iota = nc.dram_tensor("iota", [128, 128], f32, kind="ExternalInput")
    ident = nc.dram_tensor("ident", [128, 128], f32, kind="ExternalInput")
    out = nc.dram_tensor("out", [DPC, 2], f32, kind="ExternalOutput")

    eq = mybir.AluOpType.is_equal
    mult = mybir.AluOpType.mult
    subtract = mybir.AluOpType.subtract
    add = mybir.AluOpType.add
    AF = mybir.ActivationFunctionType

    with tile.TileContext(nc) as tc:
        with tc.tile_pool(name="cst", bufs=1) as cst, \
             tc.tile_pool(name="gp", bufs=8) as gp, \
             tc.tile_pool(name="sp", bufs=8) as sp, \
             tc.tile_pool(name="mp", bufs=3) as mp, \
             tc.tile_pool(name="rp", bufs=3) as rp, \
             tc.tile_pool(name="hp", bufs=4) as hp, \
             tc.tile_pool(name="pagg", bufs=2, space="PSUM") as pagg, \
             tc.tile_pool(name="ph", bufs=2, space="PSUM") as ph, \
             tc.tile_pool(name="ptr", bufs=2, space="PSUM") as ptr, \
             tc.tile_pool(name="dr", bufs=1, space="DRAM") as dr:

            def c(name, src_ap, shape, dtype=f32):
                t = cst.tile(shape, dtype, name=name)
                nc.sync.dma_start(out=t[:], in_=src_ap)
                return t

            w1l_s = c("w1l_s", w1l[:, :], [F, F])
            w1r_s = c("w1r_s", w1r[:, :], [F, F])
            w2l_s = c("w2l_s", w2l[:, :], [F, F])
            w2r_s = c("w2r_s", w2r[:, :], [F, F])
            b1_s = c("b1_s", b1[:, :], [F, 1])
            b2_s = c("b2_s", b2[:, :], [F, 1])
            wlin_s = c("wlin_s", wlin[:, :], [F, 2])
            blin_s = c("blin_s", blinb[:, :], [128, 2])
            iota_s = c("iota_s", iota[:, :], [128, 128])
            ident_s = c("ident_s", ident[:, :], [128, 128])
            xT_s = c("xT_s", xT[:, :], [128, DPC])
            g1_s = c("g1_s", g1[:, :], [128, nt], mybir.dt.int32)
            g2_s = c("g2_s", g2[:, :], [128, nt], mybir.dt.int32)
            dsl_s = c("dsl_s", dsl[:, :], [128, nt])
            wgt_s = c("wgt_s", wgt[:, :], [128, nt])
            h1T = cst.tile([128, DPC], f32, name="h1T")
            h2T = cst.tile([128, DPC], f32, name="h2T")

            ag_in = dr.tile([DPC, F], f32, name="ag_in")
            ag_out = dr.tile([NCORES * DPC, F], f32, addr_space="Shared",
                             name="ag_out")

            def sage_layer(table_ap, gidx_s, wl_s, wr_s, bias_s, inT, outT):
                for j in range(NW):
                    tw = int(t_w[j])
                    agg = pagg.tile([128, 128], f32, tag="agg")
                    for t in range(tw):
                        cc = int(col0[j]) + t
                        gb = gp.tile([128, F], f32, tag="gb")
                        nc.gpsimd.indirect_dma_start(
                            out=gb[:], out_offset=None,
                            in_=table_ap,
                            in_offset=bass.IndirectOffsetOnAxis(
                                ap=gidx_s[:, cc:cc + 1], axis=0),
                        )
                        S = sp.tile([128, 128], f32, tag="S")
                        nc.vector.tensor_scalar(
                            out=S[:], in0=iota_s[:],
                            scalar1=dsl_s[:, cc:cc + 1],
                            scalar2=wgt_s[:, cc:cc + 1],
                            op0=eq, op1=mult)
                        nc.tensor.matmul(out=agg[:], lhsT=gb[:], rhs=S[:],
                                         start=(t == 0), stop=(t == tw - 1))
                    meanT = mp.tile([128, 128], f32, tag="meanT")
                    nc.vector.tensor_copy(out=meanT[:], in_=agg[:])
                    hps = ph.tile([128, 128], f32, tag="hps")
                    nc.tensor.matmul(out=hps[:], lhsT=wl_s[:], rhs=meanT[:],
                                     start=True, stop=False)
                    nc.tensor.matmul(out=hps[:], lhsT=wr_s[:],
                                     rhs=inT[:, j * 128:(j + 1) * 128],
                                     start=False, stop=True)
                    nc.scalar.activation(
                        out=outT[:, j * 128:(j + 1) * 128], in_=hps[:],
                        func=AF.Relu, bias=bias_s[:, 0:1])

            # ---- layer 1
            sage_layer(x[:, :], g1_s, w1l_s, w1r_s, b1_s, xT_s, h1T)
            # transpose h1T windows to row layout and stage for AllGather
            for j in range(NW):
                tr = ptr.tile([128, 128], f32, tag="tr")
                nc.tensor.transpose(out=tr[:],
                                    in_=h1T[:, j * 128:(j + 1) * 128],
                                    identity=ident_s[:])
                hr = rp.tile([128, 128], f32, tag="hr")
                nc.vector.tensor_copy(out=hr[:], in_=tr[:])
                nc.sync.dma_start(out=ag_in[j * 128:(j + 1) * 128, :], in_=hr[:])
            from concourse import mybir as _mb
            nc.gpsimd.collective_compute(
                "AllGather", _mb.AluOpType.bypass,
                replica_groups=[list(range(NCORES))],
                ins=[ag_in[:, :]], outs=[ag_out[:, :]],
            )
            # ---- layer 2
            sage_layer(ag_out[:, :], g2_s, w2l_s, w2r_s, b2_s, h1T, h2T)

            # ---- head: logits + log_softmax
            for j in range(NW):
                lg = ph.tile([128, 2], f32, tag="lg")
                nc.tensor.matmul(out=lg[:], lhsT=h2T[:, j * 128:(j + 1) * 128],
                                 rhs=wlin_s[:], start=True, stop=True)
                logit = hp.tile([128, 2], f32, tag="logit")
                nc.vector.tensor_tensor(out=logit[:], in0=lg[:], in1=blin_s[:],
                                        op=add)
                negm = hp.tile([128, 1], f32, tag="negm")
                nc.vector.tensor_reduce(out=negm[:], in_=logit[:],
                                        axis=_mb.AxisListType.X,
                                        op=_mb.AluOpType.max, negate=True)
                e = hp.tile([128, 2], f32, tag="e")
                nc.scalar.activation(out=e[:], in_=logit[:], func=AF.Exp,
                                     bias=negm[:, 0:1])
                s = hp.tile([128, 1], f32, tag="s")
                nc.vector.tensor_reduce(out=s[:], in_=e[:],
                                        axis=_mb.AxisListType.X,
                                        op=_mb.AluOpType.add)
                lns = hp.tile([128, 1], f32, tag="lns")
                nc.scalar.activation(out=lns[:], in_=s[:], func=AF.Ln)
                cc_t = hp.tile([128, 1], f32, tag="cc_t")
                nc.vector.tensor_tensor(out=cc_t[:], in0=negm[:], in1=lns[:],
                                        op=subtract)
                res = hp.tile([128, 2], f32, tag="res")
                nc.vector.tensor_scalar(out=res[:], in0=logit[:],
                                        scalar1=cc_t[:, 0:1], scalar2=None,
                                        op0=add)
                nc.sync.dma_start(out=out[j * 128:(j + 1) * 128, :], in_=res[:])

    nc.compile()
    return nc


_CACHE = {}
LAST_RESULTS = None


def kernel(x, edge_index, W1l, b1, W1r, W2l, b2, W2r, Wlin, blin):
    global LAST_RESULTS
    import concourse.bass as bass  # noqa: F401  (ensures path set)
    from concourse.bass_utils import run_bass_kernel_spmd

    x = np.asarray(x, dtype=np.float32)
    ekey = np.asarray(edge_index)
    key = hash(ekey.tobytes()[:4096]) ^ hash(ekey.shape)
    if key not in _CACHE:
        pk = _pack(ekey)
        ncb = _build(pk["t_w"], pk["col0"], pk["nt"])
        _CACHE[key] = (pk, ncb)
    pk, ncb = _CACHE[key]

    core_of, win_of, slot_of = pk["core_of"], pk["win_of"], pk["slot_of"]
    # xT per core: feature-major local node features
    xT = np.zeros((NCORES, 128, DPC), dtype=np.float32)
    colidx = win_of.astype(np.int64) * 128 + slot_of
    for k in range(NCORES):
        m = core_of == k
        xT[k][:, colidx[m]] = x[m].T

    iota = np.broadcast_to(np.arange(128, dtype=np.float32), (128, 128)).copy()
    ident = np.eye(128, dtype=np.float32)
    blinb = np.broadcast_to(np.asarray(blin, np.float32), (128, 2)).copy()
    consts = dict(
        w1l=np.asarray(W1l, np.float32), w1r=np.asarray(W1r, np.float32),
        w2l=np.asarray(W2l, np.float32), w2r=np.asarray(W2r, np.float32),
        b1=np.asarray(b1, np.float32).reshape(F, 1),
        b2=np.asarray(b2, np.float32).reshape(F, 1),
        wlin=np.asarray(Wlin, np.float32), blinb=blinb,
        iota=iota, ident=ident, x=x,
    )
    in_maps = []
    for k in range(NCORES):
        in_maps.append(dict(consts, xT=xT[k], g1=pk["g1"][k], g2=pk["g2"][k],
                            dsl=pk["dsl"][k], wgt=pk["wgt"][k]))

    import os
    trace = bool(os.environ.get("KERNEL_TRACE"))
    r = run_bass_kernel_spmd(ncb, in_maps, list(range(NCORES)), trace=trace)
    LAST_RESULTS = r

    outp = np.empty((N, 2), dtype=np.float32)
    rowidx = win_of.astype(np.int64) * 128 + slot_of
    for k in range(NCORES):
        m = core_of == k
        outp[m] = r.results[k]["out"][rowidx[m]]
    return outp
